# revision 6
# baseline (speedup 1.0000x reference)
"""Trainium2 Bass kernel for nn_KNNModule_2946347565933.

Effective computation (batch/KNN collapse to a residual delta-MLP; `batch` is
unused by the reference):
    w = lrelu(bn(weights @ ri_W0)); w = lrelu(bn(w @ ri_W1))
    for l in 0..3:  h = lrelu(bn(w @ dW0[l])); d = h @ dW1[l] + db1[l]
                    pos += d[:, :2]; w += d[:, 2:]
    h = lrelu(bn(w @ ro_W0)); w_out = h @ ro_W1 + ro_b1
    return pos, w_out

v2 strategy (8 cores, data-parallel over N=400000, R=50000 rows/core):
 - channels-on-partitions; residual stream STR [128, 50000] fp16 in SBUF.
 - per-shard BN stats for layers 2..7 from a 25k-row sample per core;
   BN1 stats exact from host (2x2 second moment of `weights`).
 - "pair" processing: 2 tiles of 500 rows share one 2-bank PSUM pair so the
   big Scalar/DVE ops span 1000 columns (halves fixed overheads).
 - recompute scheme: each phase recomputes its preact matmul. BN stats for
   the NEXT layer are sampled via an accumulating matmul pair
   W_next^T x + (dW1w @ W_next)^T h  (the second factor host-precomputed),
   which keeps the stats stream OFF the DVE residual-add critical path.
 - the preact matmul for pair j+1 is emitted before pair j's consumers
   (software pipelining) so the PE never waits on Scalar/DVE.
 - dpos/wout matmuls pack both pair halves into ONE psum bank via column
   tiling (left half at partitions 0:2, right half at partitions 32:34);
   copied psum->SBUF fp16 (split between Scalar and DVE) then DMA'd;
   db1/ro_b1 and the final pos accumulation applied on host.
 - PSUM budget (8 banks): a-pair x2 bufs (4) + u-pair (2) + stats (1) +
   dpos/wout (1).
 - PE is kept dense (back-to-back matmuls) so the HAM clock-gate reaches
   2.4 GHz instead of the baseline's 1.2 GHz.
"""
import os
import sys

sys.path.insert(0, "/opt/trn_rl_repo")

from contextlib import ExitStack

import numpy as np

import concourse.bass as bass
import concourse.bacc as bacc
import concourse.mybir as mybir
import concourse.tile as tile
from concourse.bass_utils import run_bass_kernel_spmd

F32 = mybir.dt.float32
F16 = mybir.dt.float16

NCORES = 8
N, D, C_IN, H, C_OUT, L = 400000, 2, 2, 128, 2, 4
R = N // NCORES          # rows per core
TF = 500                 # tile free size (rows per tile)
PF = 2 * TF              # pair free size
NP = R // PF             # pairs per pass (50)
HB = 512                 # psum bank stride in f32 elements
EPS = 1e-5
SLOPE = 0.01

_cache = {}


def _install_trace_hook():
    """Recreate the missing antenv.axon_hooks NTFF-profile hook via ctypes so
    run_bass_kernel_spmd(trace=True) can capture device profiles under axon."""
    import types

    if "antenv.axon_hooks" not in sys.modules:
        mod = types.ModuleType("antenv.axon_hooks")
        mod._h = None
        mod.set_axon_ntff_profile_hook = lambda h: setattr(mod, "_h", h)
        mod.get_axon_ntff_profile_hook = lambda: mod._h
        sys.modules["antenv.axon_hooks"] = mod
        import antenv

        antenv.axon_hooks = mod
    from antenv.axon_hooks import (
        get_axon_ntff_profile_hook,
        set_axon_ntff_profile_hook,
    )

    if get_axon_ntff_profile_hook() is None:
        if "/root/.axon_site" not in sys.path:
            sys.path.insert(0, "/root/.axon_site")
        from trn_agent_boot.trn_boot import _ntff_profile_via_ctypes

        set_axon_ntff_profile_hook(
            _ntff_profile_via_ctypes("/opt/axon/libaxon_pjrt.so"))
    import concourse.bass_utils as bu

    bu.upload_artifacts = lambda tmpdir: "local://" + tmpdir


def _pair(tile_, parts=H):
    """AP spanning both bank halves of a [*, 2*HB] psum pair tile as
    [parts, 2, TF]."""
    return bass.AP(tensor=tile_.tensor, offset=tile_.offset,
                   ap=[[tile_.ap[0][0], parts], [HB, 2], [1, TF]])


def _sb2(ap_, cols=TF, parts=H):
    """View of a contiguous SBUF region [parts, 2*cols] as [parts, 2, cols]
    so shapes match _pair()."""
    return bass.AP(tensor=ap_.tensor, offset=ap_.offset,
                   ap=[[ap_.ap[0][0], parts], [cols, 2], [1, cols]])


def _build():
    nc = bacc.Bacc("TRN2", target_bir_lowering=False, debug=False,
                   num_devices=NCORES)
    # ---- I/O ----
    w0t_d = nc.dram_tensor("w0t", [C_IN, R], F16, kind="ExternalInput")
    riW0_d = nc.dram_tensor("riW0", [C_IN, H], F16, kind="ExternalInput")
    riW1_d = nc.dram_tensor("riW1", [H, H], F16, kind="ExternalInput")
    dW0_d = nc.dram_tensor("dW0", [L, H, H], F16, kind="ExternalInput")
    dW1w_d = nc.dram_tensor("dW1w", [L, H, H], F16, kind="ExternalInput")
    dW1p_d = nc.dram_tensor("dW1p", [L, H, D], F16, kind="ExternalInput")
    # nxtH[l] = dW1w[l] @ W_next[l]  (host-precomputed stats helper)
    nxtH_d = nc.dram_tensor("nxtH", [L, H, H], F16, kind="ExternalInput")
    roW0_d = nc.dram_tensor("roW0", [H, H], F16, kind="ExternalInput")
    roW1_d = nc.dram_tensor("roW1", [H, C_OUT], F16, kind="ExternalInput")
    # per-partition BN params: col k = BN layer k+2 (layers 2..7)
    g_d = nc.dram_tensor("gT", [H, 6], F32, kind="ExternalInput")
    be_d = nc.dram_tensor("beT", [H, 6], F32, kind="ExternalInput")
    s1t1_d = nc.dram_tensor("s1t1", [H, 2], F32, kind="ExternalInput")

    dpos_d = nc.dram_tensor("dpos", [L, D, R], F16, kind="ExternalOutput")
    wout_d = nc.dram_tensor("wout", [C_OUT, R], F16, kind="ExternalOutput")

    with tile.TileContext(nc) as tc, ExitStack() as ctx:
        P = H
        sb = ctx.enter_context(tc.tile_pool(name="sb", bufs=1))
        hpool = ctx.enter_context(tc.tile_pool(name="hp", bufs=2))
        dpst = ctx.enter_context(tc.tile_pool(name="dpst", bufs=4))
        w0pool = ctx.enter_context(tc.tile_pool(name="w0p", bufs=4))
        recp = ctx.enter_context(tc.tile_pool(name="recp", bufs=2))
        stp = ctx.enter_context(tc.tile_pool(name="stp", bufs=4))
        smalls = ctx.enter_context(tc.tile_pool(name="smalls", bufs=2))
        # psum (8 banks): a-pair x2 (4) + u-pair (2) + stats an (1) + dp (1)
        pa = ctx.enter_context(tc.tile_pool(name="pa", bufs=2, space="PSUM"))
        pu = ctx.enter_context(tc.tile_pool(name="pu", bufs=1, space="PSUM"))
        pan = ctx.enter_context(tc.tile_pool(name="pan", bufs=1, space="PSUM"))
        pd = ctx.enter_context(tc.tile_pool(name="pd", bufs=1, space="PSUM"))

        # ---- resident SBUF tensors ----
        STR = sb.tile([P, R], F16, tag="STR")
        riW0 = sb.tile([C_IN, H], F16, tag="riW0")
        riW1 = sb.tile([H, H], F16, tag="riW1")
        dW0 = [sb.tile([H, H], F16, tag=f"dW0_{l}", name=f"dW0_{l}")
               for l in range(L)]
        dW1w = [sb.tile([H, H], F16, tag=f"dW1w_{l}", name=f"dW1w_{l}")
                for l in range(L)]
        dW1p = [sb.tile([H, D], F16, tag=f"dW1p_{l}", name=f"dW1p_{l}")
                for l in range(L)]
        nxtH = [sb.tile([H, H], F16, tag=f"nxtH_{l}", name=f"nxtH_{l}")
                for l in range(L)]
        roW0 = sb.tile([H, H], F16, tag="roW0")
        roW1 = sb.tile([H, C_OUT], F16, tag="roW1")
        gT = sb.tile([H, 6], F32, tag="gT")
        beT = sb.tile([H, 6], F32, tag="beT")
        s1t1 = sb.tile([H, 2], F32, tag="s1t1")
        epst = sb.tile([H, 1], F32, tag="epst")

        nc.sync.dma_start(out=riW0, in_=riW0_d.ap())
        nc.sync.dma_start(out=riW1, in_=riW1_d.ap())
        for l in range(L):
            nc.sync.dma_start(out=dW0[l], in_=dW0_d.ap()[l])
            nc.sync.dma_start(out=dW1p[l], in_=dW1p_d.ap()[l])
            nc.sync.dma_start(out=dW1w[l], in_=dW1w_d.ap()[l])
            nc.sync.dma_start(out=nxtH[l], in_=nxtH_d.ap()[l])
        nc.sync.dma_start(out=roW0, in_=roW0_d.ap())
        nc.sync.dma_start(out=roW1, in_=roW1_d.ap())
        nc.sync.dma_start(out=gT, in_=g_d.ap())
        nc.sync.dma_start(out=beT, in_=be_d.ap())
        nc.sync.dma_start(out=s1t1, in_=s1t1_d.ap())
        nc.vector.memset(epst, EPS)

        def stats_local(rec, k):
            """rec [P, NP, 6] local bn_stats records -> (s, t) for BN layer
            k+2 from this shard's sampled statistics."""
            mv = smalls.tile([P, 2], F32, tag="mv")
            nc.vector.bn_aggr(out=mv, in_=rec[:])
            s = stp.tile([P, 1], F32, tag="s")
            t = stp.tile([P, 1], F32, tag="t")
            nc.scalar.activation(out=s, in_=mv[:, 1:2],
                                 func=mybir.ActivationFunctionType.Sqrt,
                                 bias=epst[:], scale=1.0)
            nc.vector.reciprocal(out=s, in_=s)
            nc.vector.tensor_mul(out=s, in0=s, in1=gT[:, k:k + 1])
            nc.vector.tensor_mul(out=t, in0=mv[:, 0:1], in1=s)
            nc.vector.tensor_sub(out=t, in0=beT[:, k:k + 1], in1=t)
            return s, t

        ts = bass.ts
        LR = mybir.ActivationFunctionType.Lrelu

        # of every 16 dpos copy-instructions, this many go to DVE
        DPV = 5

        def half(j, h_):
            return STR[:, j * PF + h_ * TF: j * PF + (h_ + 1) * TF]

        def pairmm(out_t, lhsT, rhs_l, rhs_r):
            nc.tensor.matmul(out=out_t[:, 0:TF], lhsT=lhsT, rhs=rhs_l,
                             start=True, stop=True)
            nc.tensor.matmul(out=out_t[:, HB:HB + TF], lhsT=lhsT,
                             rhs=rhs_r, start=True, stop=True)

        # ---- PH1: x1 = act(riW0^T w0) [host stats]; stats(riW1^T x1) ----
        rec = recp.tile([P, NP, 6], F32, tag="rec")
        w0s = {}
        As = {}

        def ph1_dma(j):
            w0s[j] = w0pool.tile([C_IN, PF], F16, tag="w0", name=f"w0_{j}")
            nc.sync.dma_start(out=w0s[j], in_=w0t_d.ap()[:, ts(j, PF)])

        def ph1_a(j):
            w0 = w0s[j]
            As[j] = pa.tile([P, 2 * HB], F32, tag="apair", name=f"a_{j}")
            pairmm(As[j], riW0[:], w0[:, 0:TF], w0[:, TF:PF])

        ph1_dma(0)
        ph1_dma(1)
        ph1_a(0)
        for j in range(NP):
            if j + 2 < NP:
                ph1_dma(j + 2)
            if j + 1 < NP:
                ph1_a(j + 1)
            a = As.pop(j)
            w0s.pop(j, None)
            nc.scalar.activation(out=_sb2(STR[:, ts(j, PF)]), in_=_pair(a),
                                 func=LR, bias=s1t1[:, 1:2],
                                 scale=s1t1[:, 0:1], alpha=SLOPE)
            an = pan.tile([P, HB], F32, tag="anb")
            nc.tensor.matmul(out=an[:, 0:TF], lhsT=riW1[:], rhs=half(j, 0),
                             start=True, stop=True)
            nc.vector.bn_stats(out=rec[:, j, :], in_=an[:, 0:TF])
        s, t = stats_local(rec, 0)

        # ---- PH2: recompute a2; x2 = act(a2); stats(dW0[0]^T x2) ----
        rec = recp.tile([P, NP, 6], F32, tag="rec")

        def ph2_a(j):
            As[j] = pa.tile([P, 2 * HB], F32, tag="apair", name=f"a_{j}")
            pairmm(As[j], riW1[:], half(j, 0), half(j, 1))

        ph2_a(0)
        for j in range(NP):
            if j + 1 < NP:
                ph2_a(j + 1)
            a = As.pop(j)
            nc.scalar.activation(out=_sb2(STR[:, ts(j, PF)]), in_=_pair(a),
                                 func=LR, bias=t[:], scale=s[:], alpha=SLOPE)
            an = pan.tile([P, HB], F32, tag="anb")
            nc.tensor.matmul(out=an[:, 0:TF], lhsT=dW0[0][:], rhs=half(j, 0),
                             start=True, stop=True)
            nc.vector.bn_stats(out=rec[:, j, :], in_=an[:, 0:TF])
        s, t = stats_local(rec, 1)

        # ---- PH3..PH6: blocks (recompute preact) ----
        for l in range(L):
            rec = recp.tile([P, NP, 6], F32, tag="rec")
            nxt = roW0 if l == L - 1 else dW0[l + 1]

            def blk_a(j, l=l):
                As[j] = pa.tile([P, 2 * HB], F32, tag="apair", name=f"a_{j}")
                pairmm(As[j], dW0[l][:], half(j, 0), half(j, 1))

            blk_a(0)
            for j in range(NP):
                if j + 1 < NP:
                    blk_a(j + 1)
                a = As.pop(j)
                h = hpool.tile([P, PF], F16, tag="h")
                nc.scalar.activation(out=_sb2(h[:, 0:PF]), in_=_pair(a),
                                     func=LR, bias=t[:], scale=s[:],
                                     alpha=SLOPE)
                u = pu.tile([P, 2 * HB], F32, tag="upair")
                pairmm(u, dW1w[l][:], h[:, 0:TF], h[:, TF:PF])
                # dpos: both halves into one bank via column tiling
                dp = pd.tile([P, HB], F32, tag="dpb")
                nc.tensor.matmul(out=dp[0:D, 0:TF], lhsT=dW1p[l][:],
                                 rhs=h[:, 0:TF], start=True, stop=True)
                nc.tensor.matmul(out=dp[32:32 + D, 0:TF], lhsT=dW1p[l][:],
                                 rhs=h[:, TF:PF], start=True, stop=True)
                # stats for next layer: nxt^T x_next = nxt^T x + nxtH^T h
                # (reads pre-add STR, so independent of the DVE add below)
                an = pan.tile([P, HB], F32, tag="anb")
                nc.tensor.matmul(out=an[:, 0:TF], lhsT=nxt[:], rhs=half(j, 0),
                                 start=True, stop=False)
                nc.tensor.matmul(out=an[:, 0:TF], lhsT=nxtH[l][:],
                                 rhs=h[:, 0:TF], start=False, stop=True)
                nc.vector.tensor_add(out=_sb2(STR[:, ts(j, PF)]),
                                     in0=_sb2(STR[:, ts(j, PF)]),
                                     in1=_pair(u))
                nc.vector.bn_stats(out=rec[:, j, :], in_=an[:, 0:TF])
                dps = dpst.tile([32 + D, TF], F16, tag="dps")
                if j % 16 < DPV:
                    nc.vector.tensor_copy(out=dps[0:D, :], in_=dp[0:D, 0:TF])
                    nc.vector.tensor_copy(out=dps[32:32 + D, :],
                                          in_=dp[32:32 + D, 0:TF])
                else:
                    nc.scalar.copy(out=dps[0:D, :], in_=dp[0:D, 0:TF])
                    nc.scalar.copy(out=dps[32:32 + D, :],
                                   in_=dp[32:32 + D, 0:TF])
                nc.sync.dma_start(out=dpos_d.ap()[l, :, ts(2 * j, TF)],
                                  in_=dps[0:D, :])
                nc.sync.dma_start(out=dpos_d.ap()[l, :, ts(2 * j + 1, TF)],
                                  in_=dps[32:32 + D, :])
            s, t = stats_local(rec, 2 + l)

        # ---- PH7: readout (recompute p_ro from x4) ----
        def ph7_a(j):
            As[j] = pa.tile([P, 2 * HB], F32, tag="apair", name=f"a_{j}")
            pairmm(As[j], roW0[:], half(j, 0), half(j, 1))

        ph7_a(0)
        for j in range(NP):
            if j + 1 < NP:
                ph7_a(j + 1)
            a = As.pop(j)
            h = hpool.tile([P, PF], F16, tag="h")
            nc.scalar.activation(out=_sb2(h[:, 0:PF]), in_=_pair(a), func=LR,
                                 bias=t[:], scale=s[:], alpha=SLOPE)
            o = pd.tile([P, HB], F32, tag="dpb")
            nc.tensor.matmul(out=o[0:C_OUT, 0:TF], lhsT=roW1[:],
                             rhs=h[:, 0:TF], start=True, stop=True)
            nc.tensor.matmul(out=o[32:32 + C_OUT, 0:TF], lhsT=roW1[:],
                             rhs=h[:, TF:PF], start=True, stop=True)
            os_ = dpst.tile([32 + C_OUT, TF], F16, tag="dps")
            if j % 2 == 0:
                nc.vector.tensor_copy(out=os_[0:C_OUT, :],
                                      in_=o[0:C_OUT, 0:TF])
                nc.vector.tensor_copy(out=os_[32:32 + C_OUT, :],
                                      in_=o[32:32 + C_OUT, 0:TF])
            else:
                nc.scalar.copy(out=os_[0:C_OUT, :], in_=o[0:C_OUT, 0:TF])
                nc.scalar.copy(out=os_[32:32 + C_OUT, :],
                               in_=o[32:32 + C_OUT, 0:TF])
            nc.sync.dma_start(out=wout_d.ap()[:, ts(2 * j, TF)],
                              in_=os_[0:C_OUT, :])
            nc.sync.dma_start(out=wout_d.ap()[:, ts(2 * j + 1, TF)],
                              in_=os_[32:32 + C_OUT, :])

    nc.compile()
    return nc


def kernel(positions, weights, batch,
           ri_W0, ri_b0, ri_g0, ri_be0, ri_W1, ri_b1, ri_g1, ri_be1,
           dW0, db0, dg0, dbe0, dW1, db1,
           ro_W0, ro_b0, ro_g0, ro_be0, ro_W1, ro_b1):
    positions = np.asarray(positions, np.float32)
    weights = np.asarray(weights, np.float32)

    if "nc" not in _cache:
        _cache["nc"] = _build()
    nc = _cache["nc"]

    bf = lambda x: np.asarray(x, np.float32).astype(np.float16)

    # host: exact L1 BN stats from the 2x2 second moment of `weights`
    # (linear bias ri_b0 cancels inside BN)
    w64 = weights.astype(np.float64)
    m1 = w64.mean(0)                       # [2]
    m2 = (w64.T @ w64) / N                 # [2,2]
    W0r = bf(ri_W0).astype(np.float64)
    mu1 = m1 @ W0r
    e2 = np.einsum("kc,kl,lc->c", W0r, m2, W0r)
    var1 = e2 - mu1 * mu1
    s1 = np.asarray(ri_g0, np.float64) / np.sqrt(var1 + EPS)
    t1 = np.asarray(ri_be0, np.float64) - mu1 * s1
    s1t1 = np.stack([s1, t1], 1).astype(np.float32)   # [128, 2]

    gT = np.stack([ri_g1, dg0[0], dg0[1], dg0[2], dg0[3], ro_g0], 1)
    beT = np.stack([ri_be1, dbe0[0], dbe0[1], dbe0[2], dbe0[3], ro_be0], 1)

    dW1 = np.asarray(dW1, np.float32)
    dW1w_np = bf(np.ascontiguousarray(dW1[:, :, D:]))
    dW0_np = bf(dW0)
    roW0_np = bf(ro_W0)
    # stats helper: nxtH[l] = dW1w[l] @ W_next  (W_next = dW0[l+1] or roW0)
    nxtH = np.empty((L, H, H), np.float16)
    for l in range(L):
        wn = dW0_np[l + 1] if l < L - 1 else roW0_np
        nxtH[l] = (dW1w_np[l].astype(np.float32)
                   @ wn.astype(np.float32)).astype(np.float16)

    shared = dict(
        riW0=bf(ri_W0), riW1=bf(ri_W1),
        dW0=dW0_np, dW1w=dW1w_np,
        dW1p=bf(np.ascontiguousarray(dW1[:, :, :D])),
        nxtH=nxtH,
        roW0=roW0_np, roW1=bf(ro_W1),
        gT=np.asarray(gT, np.float32), beT=np.asarray(beT, np.float32),
        s1t1=s1t1,
    )
    in_maps = []
    for c in range(NCORES):
        sl = weights[c * R:(c + 1) * R]
        in_maps.append(dict(shared, w0t=bf(np.ascontiguousarray(sl.T))))

    trace = bool(int(os.environ.get("KERNEL_TRACE", "0")))
    kw = {}
    if trace:
        _install_trace_hook()
        kw["tmpdir"] = os.environ.get("KERNEL_TRACE_DIR") or None
    res = run_bass_kernel_spmd(
        nc, in_maps, core_ids=list(range(NCORES)), trace=trace, **kw,
    )
    _cache["last_results"] = res

    # assemble
    pos = positions.astype(np.float64)
    db1 = np.asarray(db1, np.float64)
    wout = np.empty((N, C_OUT), np.float32)
    dsum = np.zeros((N, D), np.float64)
    for c in range(NCORES):
        r = res.results[c]
        dsum[c * R:(c + 1) * R] += r["dpos"].astype(np.float64).sum(0).T
        wout[c * R:(c + 1) * R] = r["wout"].T
    pos = pos + dsum + db1[:, :D].sum(0)
    wout = (wout.astype(np.float64) + np.asarray(ro_b1, np.float64)).astype(np.float32)
    return pos.astype(np.float32), wout


# revision 7
# speedup vs baseline: 1.1395x; 1.1395x over previous
"""Trainium2 Bass kernel for nn_KNNModule_2946347565933.

Effective computation (batch/KNN collapse to a residual delta-MLP; `batch` is
unused by the reference):
    w = lrelu(bn(weights @ ri_W0)); w = lrelu(bn(w @ ri_W1))
    for l in 0..3:  h = lrelu(bn(w @ dW0[l])); d = h @ dW1[l] + db1[l]
                    pos += d[:, :2]; w += d[:, 2:]
    h = lrelu(bn(w @ ro_W0)); w_out = h @ ro_W1 + ro_b1
    return pos, w_out

v2 strategy (8 cores, data-parallel over N=400000, R=50000 rows/core):
 - channels-on-partitions; residual stream STR [128, 50000] fp16 in SBUF.
 - per-shard BN stats for layers 2..7 from a 25k-row sample per core;
   BN1 stats exact from host (2x2 second moment of `weights`).
 - "pair" processing: 2 tiles of 500 rows share one 2-bank PSUM pair so the
   big Scalar/DVE ops span 1000 columns (halves fixed overheads).
 - recompute scheme: each phase recomputes its preact matmul. BN stats for
   the NEXT layer are sampled via an accumulating matmul pair
   W_next^T x + (dW1w @ W_next)^T h  (the second factor host-precomputed),
   which keeps the stats stream OFF the DVE residual-add critical path.
 - the preact matmul for pair j+1 is emitted before pair j's consumers
   (software pipelining) so the PE never waits on Scalar/DVE.
 - dpos/wout matmuls pack both pair halves into ONE psum bank via column
   tiling (left half at partitions 0:2, right half at partitions 32:34);
   copied psum->SBUF fp16 (split between Scalar and DVE) then DMA'd;
   db1/ro_b1 and the final pos accumulation applied on host.
 - PSUM budget (8 banks): a-pair x2 bufs (4) + u-pair (2) + stats (1) +
   dpos/wout (1).
 - PE is kept dense (back-to-back matmuls) so the HAM clock-gate reaches
   2.4 GHz instead of the baseline's 1.2 GHz.
"""
import os
import sys

sys.path.insert(0, "/opt/trn_rl_repo")

from contextlib import ExitStack

import numpy as np

import concourse.bass as bass
import concourse.bacc as bacc
import concourse.mybir as mybir
import concourse.tile as tile
from concourse.bass_utils import run_bass_kernel_spmd

F32 = mybir.dt.float32
F16 = mybir.dt.float16

NCORES = 8
N, D, C_IN, H, C_OUT, L = 400000, 2, 2, 128, 2, 4
R = N // NCORES          # rows per core
TF = 500                 # tile free size (rows per tile)
PF = 2 * TF              # pair free size
NP = R // PF             # pairs per pass (50)
HB = 512                 # psum bank stride in f32 elements
EPS = 1e-5
SLOPE = 0.01

_cache = {}


def _install_trace_hook():
    """Recreate the missing antenv.axon_hooks NTFF-profile hook via ctypes so
    run_bass_kernel_spmd(trace=True) can capture device profiles under axon."""
    import types

    if "antenv.axon_hooks" not in sys.modules:
        mod = types.ModuleType("antenv.axon_hooks")
        mod._h = None
        mod.set_axon_ntff_profile_hook = lambda h: setattr(mod, "_h", h)
        mod.get_axon_ntff_profile_hook = lambda: mod._h
        sys.modules["antenv.axon_hooks"] = mod
        import antenv

        antenv.axon_hooks = mod
    from antenv.axon_hooks import (
        get_axon_ntff_profile_hook,
        set_axon_ntff_profile_hook,
    )

    if get_axon_ntff_profile_hook() is None:
        if "/root/.axon_site" not in sys.path:
            sys.path.insert(0, "/root/.axon_site")
        from trn_agent_boot.trn_boot import _ntff_profile_via_ctypes

        set_axon_ntff_profile_hook(
            _ntff_profile_via_ctypes("/opt/axon/libaxon_pjrt.so"))
    import concourse.bass_utils as bu

    bu.upload_artifacts = lambda tmpdir: "local://" + tmpdir


def _pair(tile_, parts=H):
    """AP spanning both bank halves of a [*, 2*HB] psum pair tile as
    [parts, 2, TF]."""
    return bass.AP(tensor=tile_.tensor, offset=tile_.offset,
                   ap=[[tile_.ap[0][0], parts], [HB, 2], [1, TF]])


def _sb2(ap_, cols=TF, parts=H):
    """View of a contiguous SBUF region [parts, 2*cols] as [parts, 2, cols]
    so shapes match _pair()."""
    return bass.AP(tensor=ap_.tensor, offset=ap_.offset,
                   ap=[[ap_.ap[0][0], parts], [cols, 2], [1, cols]])


def _build():
    nc = bacc.Bacc("TRN2", target_bir_lowering=False, debug=False,
                   num_devices=NCORES)
    # ---- I/O ----
    w0t_d = nc.dram_tensor("w0t", [C_IN, R], F16, kind="ExternalInput")
    riW0_d = nc.dram_tensor("riW0", [C_IN, H], F16, kind="ExternalInput")
    riW1_d = nc.dram_tensor("riW1", [H, H], F16, kind="ExternalInput")
    dW0_d = nc.dram_tensor("dW0", [L, H, H], F16, kind="ExternalInput")
    dW1w_d = nc.dram_tensor("dW1w", [L, H, H], F16, kind="ExternalInput")
    dW1p_d = nc.dram_tensor("dW1p", [L, H, D], F16, kind="ExternalInput")
    # nxtH[l] = dW1w[l] @ W_next[l]  (host-precomputed stats helper)
    nxtH_d = nc.dram_tensor("nxtH", [L, H, H], F16, kind="ExternalInput")
    roW0_d = nc.dram_tensor("roW0", [H, H], F16, kind="ExternalInput")
    roW1_d = nc.dram_tensor("roW1", [H, C_OUT], F16, kind="ExternalInput")
    # per-partition BN params: col k = BN layer k+2 (layers 2..7)
    g_d = nc.dram_tensor("gT", [H, 6], F32, kind="ExternalInput")
    be_d = nc.dram_tensor("beT", [H, 6], F32, kind="ExternalInput")
    s1t1_d = nc.dram_tensor("s1t1", [H, 2], F32, kind="ExternalInput")

    dpos_d = nc.dram_tensor("dpos", [L, D, R], F16, kind="ExternalOutput")
    wout_d = nc.dram_tensor("wout", [C_OUT, R], F16, kind="ExternalOutput")

    with tile.TileContext(nc) as tc, ExitStack() as ctx:
        P = H
        sb = ctx.enter_context(tc.tile_pool(name="sb", bufs=1))
        hpool = ctx.enter_context(tc.tile_pool(name="hp", bufs=2))
        dpst = ctx.enter_context(tc.tile_pool(name="dpst", bufs=4))
        w0pool = ctx.enter_context(tc.tile_pool(name="w0p", bufs=4))
        recp = ctx.enter_context(tc.tile_pool(name="recp", bufs=2))
        stp = ctx.enter_context(tc.tile_pool(name="stp", bufs=4))
        smalls = ctx.enter_context(tc.tile_pool(name="smalls", bufs=2))
        # psum (8 banks): a-pair x2 (4) + u-pair (2) + stats an (1) + dp (1)
        pa = ctx.enter_context(tc.tile_pool(name="pa", bufs=2, space="PSUM"))
        pu = ctx.enter_context(tc.tile_pool(name="pu", bufs=1, space="PSUM"))
        pan = ctx.enter_context(tc.tile_pool(name="pan", bufs=1, space="PSUM"))
        pd = ctx.enter_context(tc.tile_pool(name="pd", bufs=1, space="PSUM"))

        # ---- resident SBUF tensors ----
        STR = sb.tile([P, R], F16, tag="STR")
        riW0 = sb.tile([C_IN, H], F16, tag="riW0")
        riW1 = sb.tile([H, H], F16, tag="riW1")
        dW0 = [sb.tile([H, H], F16, tag=f"dW0_{l}", name=f"dW0_{l}")
               for l in range(L)]
        dW1w = [sb.tile([H, H], F16, tag=f"dW1w_{l}", name=f"dW1w_{l}")
                for l in range(L)]
        dW1p = [sb.tile([H, D], F16, tag=f"dW1p_{l}", name=f"dW1p_{l}")
                for l in range(L)]
        nxtH = [sb.tile([H, H], F16, tag=f"nxtH_{l}", name=f"nxtH_{l}")
                for l in range(L)]
        roW0 = sb.tile([H, H], F16, tag="roW0")
        roW1 = sb.tile([H, C_OUT], F16, tag="roW1")
        gT = sb.tile([H, 6], F32, tag="gT")
        beT = sb.tile([H, 6], F32, tag="beT")
        s1t1 = sb.tile([H, 2], F32, tag="s1t1")
        epst = sb.tile([H, 1], F32, tag="epst")

        nc.sync.dma_start(out=riW0, in_=riW0_d.ap())
        nc.sync.dma_start(out=riW1, in_=riW1_d.ap())
        for l in range(L):
            nc.sync.dma_start(out=dW0[l], in_=dW0_d.ap()[l])
            nc.sync.dma_start(out=dW1p[l], in_=dW1p_d.ap()[l])
            nc.sync.dma_start(out=dW1w[l], in_=dW1w_d.ap()[l])
            nc.sync.dma_start(out=nxtH[l], in_=nxtH_d.ap()[l])
        nc.sync.dma_start(out=roW0, in_=roW0_d.ap())
        nc.sync.dma_start(out=roW1, in_=roW1_d.ap())
        nc.sync.dma_start(out=gT, in_=g_d.ap())
        nc.sync.dma_start(out=beT, in_=be_d.ap())
        nc.sync.dma_start(out=s1t1, in_=s1t1_d.ap())
        nc.vector.memset(epst, EPS)

        def stats_local(rec, k):
            """rec [P, NP, 6] local bn_stats records -> (s, t) for BN layer
            k+2 from this shard's sampled statistics."""
            mv = smalls.tile([P, 2], F32, tag="mv")
            nc.vector.bn_aggr(out=mv, in_=rec[:])
            s = stp.tile([P, 1], F32, tag="s")
            t = stp.tile([P, 1], F32, tag="t")
            nc.scalar.activation(out=s, in_=mv[:, 1:2],
                                 func=mybir.ActivationFunctionType.Sqrt,
                                 bias=epst[:], scale=1.0)
            nc.vector.reciprocal(out=s, in_=s)
            nc.vector.tensor_mul(out=s, in0=s, in1=gT[:, k:k + 1])
            nc.vector.tensor_mul(out=t, in0=mv[:, 0:1], in1=s)
            nc.vector.tensor_sub(out=t, in0=beT[:, k:k + 1], in1=t)
            return s, t

        ts = bass.ts
        LR = mybir.ActivationFunctionType.Prelu

        # of every 16 dpos copy-instructions, this many go to DVE
        DPV = 8

        def half(j, h_):
            return STR[:, j * PF + h_ * TF: j * PF + (h_ + 1) * TF]

        def pairmm(out_t, lhsT, rhs_l, rhs_r):
            nc.tensor.matmul(out=out_t[:, 0:TF], lhsT=lhsT, rhs=rhs_l,
                             start=True, stop=True)
            nc.tensor.matmul(out=out_t[:, HB:HB + TF], lhsT=lhsT,
                             rhs=rhs_r, start=True, stop=True)

        # ---- PH1: x1 = act(riW0^T w0) [host stats]; stats(riW1^T x1) ----
        rec = recp.tile([P, NP, 6], F32, tag="rec")
        w0s = {}
        As = {}

        def ph1_dma(j):
            w0s[j] = w0pool.tile([C_IN, PF], F16, tag="w0", name=f"w0_{j}")
            nc.sync.dma_start(out=w0s[j], in_=w0t_d.ap()[:, ts(j, PF)])

        def ph1_a(j):
            w0 = w0s[j]
            As[j] = pa.tile([P, 2 * HB], F32, tag="apair", name=f"a_{j}")
            pairmm(As[j], riW0[:], w0[:, 0:TF], w0[:, TF:PF])

        ph1_dma(0)
        ph1_dma(1)
        ph1_a(0)
        for j in range(NP):
            if j + 2 < NP:
                ph1_dma(j + 2)
            if j + 1 < NP:
                ph1_a(j + 1)
            a = As.pop(j)
            w0s.pop(j, None)
            nc.scalar.activation(out=_sb2(STR[:, ts(j, PF)]), in_=_pair(a),
                                 func=LR, bias=s1t1[:, 1:2],
                                 scale=s1t1[:, 0:1], alpha=SLOPE)
            an = pan.tile([P, HB], F32, tag="anb")
            nc.tensor.matmul(out=an[:, 0:TF], lhsT=riW1[:], rhs=half(j, 0),
                             start=True, stop=True)
            nc.vector.bn_stats(out=rec[:, j, :], in_=an[:, 0:TF])
        s, t = stats_local(rec, 0)

        # ---- PH2: recompute a2; x2 = act(a2); stats(dW0[0]^T x2) ----
        rec = recp.tile([P, NP, 6], F32, tag="rec")

        def ph2_a(j):
            As[j] = pa.tile([P, 2 * HB], F32, tag="apair", name=f"a_{j}")
            pairmm(As[j], riW1[:], half(j, 0), half(j, 1))

        ph2_a(0)
        for j in range(NP):
            if j + 1 < NP:
                ph2_a(j + 1)
            a = As.pop(j)
            nc.scalar.activation(out=_sb2(STR[:, ts(j, PF)]), in_=_pair(a),
                                 func=LR, bias=t[:], scale=s[:], alpha=SLOPE)
            an = pan.tile([P, HB], F32, tag="anb")
            nc.tensor.matmul(out=an[:, 0:TF], lhsT=dW0[0][:], rhs=half(j, 0),
                             start=True, stop=True)
            nc.vector.bn_stats(out=rec[:, j, :], in_=an[:, 0:TF])
        s, t = stats_local(rec, 1)

        # ---- PH3..PH6: blocks (recompute preact) ----
        for l in range(L):
            rec = recp.tile([P, NP, 6], F32, tag="rec")
            nxt = roW0 if l == L - 1 else dW0[l + 1]

            def blk_a(j, l=l):
                As[j] = pa.tile([P, 2 * HB], F32, tag="apair", name=f"a_{j}")
                pairmm(As[j], dW0[l][:], half(j, 0), half(j, 1))

            blk_a(0)
            for j in range(NP):
                if j + 1 < NP:
                    blk_a(j + 1)
                a = As.pop(j)
                h = hpool.tile([P, PF], F16, tag="h")
                nc.scalar.activation(out=_sb2(h[:, 0:PF]), in_=_pair(a),
                                     func=LR, bias=t[:], scale=s[:],
                                     alpha=SLOPE)
                u = pu.tile([P, 2 * HB], F32, tag="upair")
                pairmm(u, dW1w[l][:], h[:, 0:TF], h[:, TF:PF])
                # dpos: both halves into one bank via column tiling
                dp = pd.tile([P, HB], F32, tag="dpb")
                nc.tensor.matmul(out=dp[0:D, 0:TF], lhsT=dW1p[l][:],
                                 rhs=h[:, 0:TF], start=True, stop=True)
                nc.tensor.matmul(out=dp[32:32 + D, 0:TF], lhsT=dW1p[l][:],
                                 rhs=h[:, TF:PF], start=True, stop=True)
                # stats for next layer: nxt^T x_next = nxt^T x + nxtH^T h
                # (reads pre-add STR, so independent of the DVE add below)
                an = pan.tile([P, HB], F32, tag="anb")
                nc.tensor.matmul(out=an[:, 0:TF], lhsT=nxt[:], rhs=half(j, 0),
                                 start=True, stop=False)
                nc.tensor.matmul(out=an[:, 0:TF], lhsT=nxtH[l][:],
                                 rhs=h[:, 0:TF], start=False, stop=True)
                nc.vector.bn_stats(out=rec[:, j, :], in_=an[:, 0:TF])
                nc.vector.tensor_add(out=_sb2(STR[:, ts(j, PF)]),
                                     in0=_sb2(STR[:, ts(j, PF)]),
                                     in1=_pair(u))
                dps = dpst.tile([32 + D, TF], F16, tag="dps")
                nc.scalar.copy(out=dps[0:D, :], in_=dp[0:D, 0:TF])
                if j % 16 < DPV:
                    nc.vector.tensor_copy(out=dps[32:32 + D, :],
                                          in_=dp[32:32 + D, 0:TF])
                else:
                    nc.scalar.copy(out=dps[32:32 + D, :],
                                   in_=dp[32:32 + D, 0:TF])
                nc.sync.dma_start(out=dpos_d.ap()[l, :, ts(2 * j, TF)],
                                  in_=dps[0:D, :])
                nc.sync.dma_start(out=dpos_d.ap()[l, :, ts(2 * j + 1, TF)],
                                  in_=dps[32:32 + D, :])
            s, t = stats_local(rec, 2 + l)

        # ---- PH7: readout (recompute p_ro from x4) ----
        def ph7_a(j):
            As[j] = pa.tile([P, 2 * HB], F32, tag="apair", name=f"a_{j}")
            pairmm(As[j], roW0[:], half(j, 0), half(j, 1))

        ph7_a(0)
        for j in range(NP):
            if j + 1 < NP:
                ph7_a(j + 1)
            a = As.pop(j)
            h = hpool.tile([P, PF], F16, tag="h")
            nc.scalar.activation(out=_sb2(h[:, 0:PF]), in_=_pair(a), func=LR,
                                 bias=t[:], scale=s[:], alpha=SLOPE)
            opool = pd if j % 2 == 0 else pan
            o = opool.tile([P, HB], F32, tag="dpb" if j % 2 == 0 else "anb",
                           name=f"o_{j}")
            nc.tensor.matmul(out=o[0:C_OUT, 0:TF], lhsT=roW1[:],
                             rhs=h[:, 0:TF], start=True, stop=True)
            nc.tensor.matmul(out=o[32:32 + C_OUT, 0:TF], lhsT=roW1[:],
                             rhs=h[:, TF:PF], start=True, stop=True)
            os_ = dpst.tile([32 + C_OUT, TF], F16, tag="dps")
            nc.vector.tensor_copy(out=os_[0:C_OUT, :],
                                  in_=o[0:C_OUT, 0:TF])
            nc.vector.tensor_copy(out=os_[32:32 + C_OUT, :],
                                  in_=o[32:32 + C_OUT, 0:TF])
            nc.sync.dma_start(out=wout_d.ap()[:, ts(2 * j, TF)],
                              in_=os_[0:C_OUT, :])
            nc.sync.dma_start(out=wout_d.ap()[:, ts(2 * j + 1, TF)],
                              in_=os_[32:32 + C_OUT, :])

    nc.compile()
    return nc


def kernel(positions, weights, batch,
           ri_W0, ri_b0, ri_g0, ri_be0, ri_W1, ri_b1, ri_g1, ri_be1,
           dW0, db0, dg0, dbe0, dW1, db1,
           ro_W0, ro_b0, ro_g0, ro_be0, ro_W1, ro_b1):
    positions = np.asarray(positions, np.float32)
    weights = np.asarray(weights, np.float32)

    if "nc" not in _cache:
        _cache["nc"] = _build()
    nc = _cache["nc"]

    bf = lambda x: np.asarray(x, np.float32).astype(np.float16)

    # host: exact L1 BN stats from the 2x2 second moment of `weights`
    # (linear bias ri_b0 cancels inside BN)
    w64 = weights.astype(np.float64)
    m1 = w64.mean(0)                       # [2]
    m2 = (w64.T @ w64) / N                 # [2,2]
    W0r = bf(ri_W0).astype(np.float64)
    mu1 = m1 @ W0r
    e2 = np.einsum("kc,kl,lc->c", W0r, m2, W0r)
    var1 = e2 - mu1 * mu1
    s1 = np.asarray(ri_g0, np.float64) / np.sqrt(var1 + EPS)
    t1 = np.asarray(ri_be0, np.float64) - mu1 * s1
    s1t1 = np.stack([s1, t1], 1).astype(np.float32)   # [128, 2]

    gT = np.stack([ri_g1, dg0[0], dg0[1], dg0[2], dg0[3], ro_g0], 1)
    beT = np.stack([ri_be1, dbe0[0], dbe0[1], dbe0[2], dbe0[3], ro_be0], 1)

    dW1 = np.asarray(dW1, np.float32)
    dW1w_np = bf(np.ascontiguousarray(dW1[:, :, D:]))
    dW0_np = bf(dW0)
    roW0_np = bf(ro_W0)
    # stats helper: nxtH[l] = dW1w[l] @ W_next  (W_next = dW0[l+1] or roW0)
    nxtH = np.empty((L, H, H), np.float16)
    for l in range(L):
        wn = dW0_np[l + 1] if l < L - 1 else roW0_np
        nxtH[l] = (dW1w_np[l].astype(np.float32)
                   @ wn.astype(np.float32)).astype(np.float16)

    shared = dict(
        riW0=bf(ri_W0), riW1=bf(ri_W1),
        dW0=dW0_np, dW1w=dW1w_np,
        dW1p=bf(np.ascontiguousarray(dW1[:, :, :D])),
        nxtH=nxtH,
        roW0=roW0_np, roW1=bf(ro_W1),
        gT=np.asarray(gT, np.float32), beT=np.asarray(beT, np.float32),
        s1t1=s1t1,
    )
    in_maps = []
    for c in range(NCORES):
        sl = weights[c * R:(c + 1) * R]
        in_maps.append(dict(shared, w0t=bf(np.ascontiguousarray(sl.T))))

    trace = bool(int(os.environ.get("KERNEL_TRACE", "0")))
    kw = {}
    if trace:
        _install_trace_hook()
        kw["tmpdir"] = os.environ.get("KERNEL_TRACE_DIR") or None
    res = run_bass_kernel_spmd(
        nc, in_maps, core_ids=list(range(NCORES)), trace=trace, **kw,
    )
    _cache["last_results"] = res

    # assemble
    pos = positions.astype(np.float64)
    db1 = np.asarray(db1, np.float64)
    wout = np.empty((N, C_OUT), np.float32)
    dsum = np.zeros((N, D), np.float64)
    for c in range(NCORES):
        r = res.results[c]
        dsum[c * R:(c + 1) * R] += r["dpos"].astype(np.float64).sum(0).T
        wout[c * R:(c + 1) * R] = r["wout"].T
    pos = pos + dsum + db1[:, :D].sum(0)
    wout = (wout.astype(np.float64) + np.asarray(ro_b1, np.float64)).astype(np.float32)
    return pos.astype(np.float32), wout
